# revision 39
# baseline (speedup 1.0000x reference)
"""SimCLR NT-Xent contrastive loss on 8 Trainium2 cores — collective-free.

Reference math (B=2048, D=256, T=0.5):
    zn = l2norm_rows(concat(z_i, z_j))          # [4096, 256]
    sim = zn @ zn.T / T                         # [4096, 4096]
    loss = mean_g [ log(sum_j exp(sim[g,j]) - exp(sim[g,g])) - sim[g, (g+B)%N] ]

Sharding: sim rows are sharded 512/core.  Instead of AllGather-ing the
normalized z across cores (two 64KB/core AllGathers cost ~100us+ of
per-execute collective/sync overhead each on this fabric), every core
receives the FULL z replicated — the host tiles the 2MB bf16 copy of z eight
ways, which costs H2D bandwidth once (inputs stay device-resident across
executions) and nothing per execution.  Each core also receives two 512-row
per-core shards so no on-device core-id logic is needed:

  zown = z[512c : 512c+512]            (its sim rows; just z row-sharded)
  zpos = roll(z, -2048)[512c : ...]    (positive-pair rows of those rows)

Per-core program (no collectives, no gpsimd sync; Pool only does copies):
  1. normalize all 4096 rows of zall: one big DVE square + one grouped
     tensor_reduce per 8-row-tile group (NOT per-tile scalar_tensor_tensor —
     3x cheaper), ACT exp(-0.5*ln) rsqrt, per-tile DVE scale (76ns each)
  2. PE-transpose (identity matmul) normalized tiles into znt [256, 4096];
     PSUM->SBUF drain copies alternate DVE / Pool so neither stalls the pipe
  3. per column-group g (1024 cols = its 8 row tiles), IMMEDIATELY after its
     transposes: sim matmuls lhsT x znt[., g] into [128 x 1024] PSUM tiles
     and ACT exp(2*sim) with accum_out -> per-row partial rowsums.  This
     interleaving keeps PE/ACT busy from ~6us instead of serializing
     normalize -> transpose -> matmul phases.
  4. positives: one fused DVE multiply + grouped reduce over zn_own*zn_pos
  5. tail: log(rowsum - e^2) (sim[g,g] == 1/T for unit rows) - 2*posdot,
     reduce free dim, partition-reduce with a ones-vector fp32 matmul ->
     single fp32 partial; host sums the 8 partials / 4096.
"""

import numpy as np

import concourse.bacc as bacc
import concourse.bass as bass
import concourse.masks as masks
import concourse.tile as tile
from concourse import mybir

F32 = mybir.dt.float32
BF16 = mybir.dt.bfloat16
AF = mybir.ActivationFunctionType
ALU = mybir.AluOpType
AXIS = mybir.AxisListType

B = 2048
D = 256
N = 2 * B            # 4096 total rows
NCORES = 8
RPC = N // NCORES    # 512 sim rows per core
NT = N // 128        # 32 row tiles of zall
E2 = float(np.exp(np.float32(2.0)))   # exp(sim[g,g]) = exp(1/T)
NPACK = N + 2 * RPC                   # zall + zown + zpos rows, one tensor
IN_NAMES = ("zpack",)                 # single packed ExternalInput
OUT_SHAPE = (1, 1)                    # per-core output shape


class _Bacc(bacc.Bacc):
    """Bacc that pins the activation-table pass to the one set containing
    both Ln and Exp — the default fixpoint picks per-function sets and
    thrashes 5 table loads (~6.4us of ACT) into the schedule."""

    def insert_act_table_loads(self):
        from concourse.hw_specs import get_activation_tables
        import bass_rust as _bass_rust

        has_activation = any(
            isinstance(i, mybir.InstActivation)
            for b in self.main_func.blocks
            for i in b.instructions
        )
        if not has_activation:
            return
        keep = {
            mybir.ActivationFunctionType.Ln,
            mybir.ActivationFunctionType.Exp,
        }
        tables = [
            (k, v if k == "natural_log_exp_and_others" else v - keep)
            for k, v in get_activation_tables(self.m.arch).items()
        ]
        _bass_rust.insert_act_table_loads(self, tables)


def build_nc():
    nc = _Bacc("TRN2", target_bir_lowering=False, debug=False)
    zpack = nc.dram_tensor("zpack", [NPACK, D], BF16, kind="ExternalInput").ap()
    out = nc.dram_tensor("out", [1, 1], F32, kind="ExternalOutput").ap()
    with tile.TileContext(nc) as tc:
        build_tile_program(
            tc,
            out,
            zpack[0:N],
            zpack[N : N + RPC],
            zpack[N + RPC : NPACK],
        )
    nc.compile()
    return nc


# zall tile groups per pipelined round (triangular: tiny first round so the
# first exp fires early, small last round so the trailing chain is short)
ROUNDS = (2, 6, 8, 8, 8)
NROUNDS = len(ROUNDS)


def build_tile_program(
    tc: tile.TileContext, out: bass.AP, zall: bass.AP, zown: bass.AP, zpos: bass.AP
):
    nc = tc.nc
    zall_v = zall.rearrange("(t p) d -> p t d", t=NT)   # row r = t*128 + p
    zown_v = zown.rearrange("(t p) d -> p t d", t=4)
    zpos_v = zpos.rearrange("(t p) d -> p t d", t=4)

    with (
        tc.tile_pool(name="consts", bufs=1) as consts,
        tc.tile_pool(name="zp", bufs=1) as zp,
        tc.tile_pool(name="znp", bufs=1) as znp,
        tc.tile_pool(name="zntp", bufs=1) as zntp,
        tc.tile_pool(name="sqp", bufs=2) as sqp,
        tc.tile_pool(name="smalls", bufs=1) as smalls,
        tc.tile_pool(name="scrp", bufs=2) as scrp,
        tc.tile_pool(name="trps", bufs=2, space="PSUM") as trps,
        tc.tile_pool(name="simp", bufs=3, space="PSUM") as simp,
    ):
        identity = consts.tile([128, 128], BF16, tag="identity")
        masks.make_identity(nc, identity[:, :])
        zero_col = consts.tile([128, 1], F32, tag="zero_col")
        nc.vector.memset(zero_col, 0.0)
        neg_e2 = consts.tile([128, 1], F32, tag="neg_e2")
        nc.vector.memset(neg_e2, -E2)
        ones_col = consts.tile([128, 1], F32, tag="ones_col")
        nc.vector.memset(ones_col, 1.0)
        ln2_col = consts.tile([128, 1], F32, tag="ln2_col")
        nc.vector.memset(ln2_col, float(np.log(2.0)))

        # persistent tiles
        za = zp.tile([128, NT, D], BF16, tag="za")        # full z rows
        zo = zp.tile([128, 4, D], BF16, tag="zo")         # own rows (raw!)
        zq = zp.tile([128, 4, D], BF16, tag="zq")         # positive rows
        zn = znp.tile([128, NT, D], BF16, tag="zn")       # normalized full z
        pdh = znp.tile([128, 4, D], BF16, tag="pdh")      # posdot products
        znt = zntp.tile([128, 2, N], BF16, tag="znt")     # znT, all 4096 cols
        znoT = zntp.tile([128, 2, RPC], BF16, tag="znoT")  # RAW own as cols

        ssa = smalls.tile([128, NT], F32, tag="ssa")
        lna = smalls.tile([128, NT], F32, tag="lna")
        inva = smalls.tile([128, NT], F32, tag="inva")
        sso = smalls.tile([128, 8], F32, tag="sso")       # own(0:4) pos(4:8)
        lno = smalls.tile([128, 8], F32, tag="lno")
        invo = smalls.tile([128, 8], F32, tag="invo")
        acc20 = smalls.tile([128, 4 * NROUNDS], F32, tag="acc20")
        posd4 = smalls.tile([128, 4], F32, tag="posd4")
        rowsum4 = smalls.tile([128, 4], F32, tag="rowsum4")
        tailA = smalls.tile([128, 4], F32, tag="tailA")
        tot4 = smalls.tile([128, 4], F32, tag="tot4")
        tail1 = smalls.tile([128, 1], F32, tag="tail1")
        result = smalls.tile([128, 1], F32, tag="result")

        # ---- loads, all on the SP queue (DMAs triggered from nc.scalar
        # block the ACT engine for the whole transfer — never use it).
        # Round 0's two tiles ship first so its sumsq starts at ~0.7us;
        # own/pos rows next (they gate the lhsT transposes and invo);
        # then one chunk per remaining round, just-in-time.
        r0 = ROUNDS[0]
        nc.sync.dma_start(
            out=za[:, 0:r0, :], in_=zall_v[:, 0:r0, :]
        )
        nc.sync.dma_start(out=zo, in_=zown_v)
        nc.sync.dma_start(out=zq, in_=zpos_v)
        ts0 = r0
        for gsz in ROUNDS[1:]:
            nc.sync.dma_start(
                out=za[:, ts0 : ts0 + gsz, :],
                in_=zall_v[:, ts0 : ts0 + gsz, :],
            )
            ts0 += gsz

        # ---- own rows stay RAW: transpose zo into the lhsT immediately on
        # arrival (shortest possible path to the first sim matmul, and warms
        # the PE out of its low p-state), and fold the own-row normalization
        # into the exp as a per-partition scale AP: invo2 = 2/|z_g| so
        # exp(invo2_g * (z_g . zn_j)) == exp(2 * sim[g, j]).
        psT = trps.tile([128, 2, RPC], BF16, tag="tr")
        for u in range(4):
            for h in range(2):
                nc.tensor.transpose(
                    psT[:, h, u * 128 : (u + 1) * 128],
                    zo[:, u, h * 128 : (h + 1) * 128],
                    identity,
                )
        nc.vector.tensor_copy(znoT, psT)

        def own_sumsq():
            # on ACT (Square+accum): invo is only needed as the first exp's
            # scale AP (~8us in), and keeping these 8 ops off DVE lets DVE
            # pace the zall rounds ~2.4us sooner.  Emitted after round 1's
            # rsqrt so they don't delay the round pipeline's inv chain.
            for u in range(4):
                sqo = sqp.tile([128, D], BF16, tag="sqo")
                nc.scalar.activation(
                    sqo, zo[:, u, :], AF.Square,
                    bias=zero_col, accum_out=sso[:, u : u + 1],
                )
            for u in range(4):
                sqq = sqp.tile([128, D], BF16, tag="sqo")
                nc.scalar.activation(
                    sqq, zq[:, u, :], AF.Square,
                    bias=zero_col, accum_out=sso[:, 4 + u : 5 + u],
                )
            nc.scalar.activation(lno, sso, AF.Ln, bias=zero_col)
            # invo = 2/sqrt(sumsq)  (ln2 bias folds the *2 temperature)
            nc.scalar.activation(invo, lno, AF.Exp, bias=ln2_col, scale=-0.5)

        def posdots():
            # positives on RAW rows; normalization correction in the tail
            nc.vector.tensor_tensor(out=pdh, in0=zo, in1=zq, op=ALU.mult)
            nc.vector.tensor_reduce(out=posd4, in_=pdh, axis=AXIS.X, op=ALU.add)

        # ---- pipelined rounds over zall tile groups: sumsq (DVE) ->
        # rsqrt (ACT) -> scale (Pool) -> PE transpose -> drain copy
        # (Pool/DVE alternate) -> sim matmuls (PE) -> exp+rowsum (ACT).
        # Stage A of round r+1 is emitted BEFORE stage B of round r so each
        # engine's in-order queue works a round ahead (in particular Pool's
        # scales must not convoy behind its drain copies).
        bounds = []
        ts = 0
        for gsz in ROUNDS:
            bounds.append((ts, ts + gsz))
            ts += gsz

        def stage_a(r):
            ts, te = bounds[r]
            for t in range(ts, te):
                sq = sqp.tile([128, D], BF16, tag="sq")
                nc.vector.scalar_tensor_tensor(
                    out=sq, in0=za[:, t, :], scalar=1.0, in1=za[:, t, :],
                    op0=ALU.mult, op1=ALU.mult, accum_out=ssa[:, t : t + 1],
                )
            gs = slice(ts, te)
            nc.scalar.activation(lna[:, gs], ssa[:, gs], AF.Ln, bias=zero_col)
            nc.scalar.activation(
                inva[:, gs], lna[:, gs], AF.Exp, bias=zero_col, scale=-0.5
            )
            for t in range(ts, te):
                nc.vector.tensor_scalar_mul(
                    zn[:, t, :], za[:, t, :], inva[:, t : t + 1]
                )

        def stage_b(r):
            ts, te = bounds[r]
            gsz = te - ts
            cols = 128 * gsz
            c0 = 128 * ts
            for pair, t0 in enumerate(range(ts, te, 4)):
                bsz = min(4, te - t0)  # up to 4 tiles per PSUM buf
                psA = trps.tile([128, 2, RPC], BF16, tag="tr")
                for dt in range(bsz):
                    t = t0 + dt
                    for h in range(2):
                        nc.tensor.transpose(
                            psA[:, h, dt * 128 : (dt + 1) * 128],
                            zn[:, t, h * 128 : (h + 1) * 128],
                            identity,
                        )
                # PSUM reads are DVE/ACT-only on TRN2 (Pool has no PSUM port)
                nc.vector.tensor_copy(
                    znt[:, :, t0 * 128 : (t0 + bsz) * 128],
                    psA[:, :, 0 : bsz * 128],
                )
            # sim matmuls + exp (per-partition scale = 2/|z_g|) for the group
            for rc in range(4):
                ps = simp.tile([128, 1024], F32, tag="ps")
                for h in range(2):
                    lhsT = znoT[:, h, rc * 128 : (rc + 1) * 128]
                    for q in range(0, cols, 512):
                        qs = min(512, cols - q)
                        nc.tensor.matmul(
                            ps[:, q : q + qs],
                            lhsT,
                            znt[:, h, c0 + q : c0 + q + qs],
                            start=(h == 0),
                            stop=(h == 1),
                        )
                scr = scrp.tile([128, 1024], BF16, tag="exp_scr")
                k = r * 4 + rc
                nc.scalar.activation(
                    scr[:, 0:cols],
                    ps[:, 0:cols],
                    AF.Exp,
                    bias=zero_col,
                    scale=invo[:, rc : rc + 1],
                    accum_out=acc20[:, k : k + 1],
                )

        # round 0's sumsq goes FIRST on the DVE queue (it gates the first
        # exp); own-row sumsq (ACT) lands after round 1's rsqrt; posdots
        # land last (only the tail reads them).
        stage_a(0)
        for r in range(NROUNDS):
            if r + 1 < NROUNDS:
                stage_a(r + 1)
            if r == 0:
                own_sumsq()
            stage_b(r)
        posdots()

        # ---- tail: rowsums, log(neg) - sim[g,pos], partition-reduce.
        # sim[g,pos] = 2*inv_g*inv_p*posd_raw = 0.5*invo_g*invo_p*posd_raw
        acc_v = acc20.rearrange("p (s r) -> p r s", s=NROUNDS)
        nc.vector.tensor_reduce(out=rowsum4, in_=acc_v, axis=AXIS.X, op=ALU.add)
        nc.scalar.activation(tailA, rowsum4, AF.Ln, bias=neg_e2)
        ipp = smalls.tile([128, 4], F32, tag="ipp")
        nc.vector.tensor_tensor(
            out=ipp, in0=invo[:, 0:4], in1=invo[:, 4:8], op=ALU.mult
        )
        ppp = smalls.tile([128, 4], F32, tag="ppp")
        nc.vector.tensor_tensor(out=ppp, in0=ipp, in1=posd4, op=ALU.mult)
        nc.vector.scalar_tensor_tensor(
            out=tot4, in0=ppp, scalar=-0.5, in1=tailA, op0=ALU.mult, op1=ALU.add
        )
        nc.vector.tensor_reduce(out=tail1, in_=tot4, axis=AXIS.X, op=ALU.add)
        res_ps = simp.tile([128, 1024], F32, tag="ps")
        nc.tensor.matmul(
            res_ps[0:1, 0:1], ones_col, tail1, start=True, stop=True
        )
        nc.vector.tensor_copy(result[0:1, :], res_ps[0:1, 0:1])
        nc.sync.dma_start(out=out, in_=result[0:1, :])


_NC_CACHE = None


def _get_nc():
    global _NC_CACHE
    if _NC_CACHE is None:
        _NC_CACHE = build_nc()
    return _NC_CACHE


def _to_bf16(x: np.ndarray) -> np.ndarray:
    import ml_dtypes

    return np.asarray(x).astype(ml_dtypes.bfloat16)


def _host_shards(z_i: np.ndarray, z_j: np.ndarray):
    """-> (zpack_stack [8*NPACK, D],) bf16: per core, the full z followed by
    its own 512 sim rows and their 512 positive-pair rows."""
    z = np.concatenate([np.asarray(z_i), np.asarray(z_j)], axis=0)
    zbf = _to_bf16(z)
    zpos = np.roll(zbf, -B, axis=0)
    parts = []
    for c in range(NCORES):
        parts.append(zbf)
        parts.append(zbf[c * RPC : (c + 1) * RPC])
        parts.append(zpos[c * RPC : (c + 1) * RPC])
    return (np.concatenate(parts, axis=0),)


def make_in_maps(z_i: np.ndarray, z_j: np.ndarray):
    (zpack_stack,) = _host_shards(z_i, z_j)
    return [
        {"zpack": zpack_stack[c * NPACK : (c + 1) * NPACK]}
        for c in range(NCORES)
    ]


_EXEC_CACHE = None


def _get_exec():
    """Jitted 8-core SPMD executable (fast-dispatch compiled), built once."""
    global _EXEC_CACHE
    if _EXEC_CACHE is None:
        import jax
        from jax.experimental.shard_map import shard_map
        from jax.sharding import Mesh, PartitionSpec

        from concourse import bass2jax

        nc = _get_nc()
        bass2jax.install_neuronx_cc_hook()
        assert nc.dbg_addr is None
        part_name = (
            nc.partition_id_tensor.name if nc.partition_id_tensor else None
        )
        # input order: ExternalInputs, partition id.  The "out" tensor is
        # NOT passed as a donated zeroed operand (run_bass_via_pjrt does
        # that only to guarantee zero-init for kernels that don't write
        # every element — ours fully writes out[1,1]); binding it purely
        # as the custom-call result drops one operand + per-call donation
        # bookkeeping from the dispatch path.
        in_names = list(IN_NAMES) + ([part_name] if part_name else [])
        out_avals = (jax.core.ShapedArray(OUT_SHAPE, np.float32),)

        def _body(*args):
            operands = list(args)
            if part_name is not None:
                operands.append(bass2jax.partition_id_tensor())
            outs = bass2jax._bass_exec_p.bind(
                *operands,
                out_avals=out_avals,
                in_names=tuple(in_names),
                out_names=("out",),
                lowering_input_output_aliases=(),
                sim_require_finite=True,
                sim_require_nnan=True,
                nc=nc,
            )
            return tuple(outs)

        devices = jax.devices()[:NCORES]
        mesh = Mesh(np.asarray(devices), ("core",))
        n_in = len(IN_NAMES)

        def make_jit():
            return jax.jit(
                shard_map(
                    _body,
                    mesh=mesh,
                    in_specs=(PartitionSpec("core"),) * n_in,
                    out_specs=(PartitionSpec("core"),),
                    check_rep=False,
                ),
                keep_unused=True,
            )

        import ml_dtypes

        ex_args = (np.zeros((NCORES * NPACK, D), ml_dtypes.bfloat16),)
        try:
            # fast_dispatch_compile minus the FastDispatchCompiled reclass:
            # kernel() always reads its outputs, so the never-read-output
            # safety net would only add per-call host work.
            with bass2jax._fast_dispatch_active(True):
                compiled = make_jit().lower(*ex_args).compile()
            if compiled._executable.unsafe_call.has_unordered_effects:
                raise RuntimeError("bass_effect still present")
            _EXEC_CACHE = compiled
        except Exception:
            _EXEC_CACHE = make_jit()
    return _EXEC_CACHE


def kernel(z_i: np.ndarray, z_j: np.ndarray) -> np.ndarray:
    """Full inputs in, full output out; shards sim rows across the 8 cores."""
    sharded = _get_exec()
    (zpack_stack,) = _host_shards(z_i, z_j)
    (partials,) = sharded(zpack_stack)
    return np.float32(float(np.asarray(partials).sum()) / N)


# revision 43
# speedup vs baseline: 1.0170x; 1.0170x over previous
"""SimCLR NT-Xent contrastive loss on 8 Trainium2 cores — collective-free.

Reference math (B=2048, D=256, T=0.5):
    zn = l2norm_rows(concat(z_i, z_j))          # [4096, 256]
    sim = zn @ zn.T / T                         # [4096, 4096]
    loss = mean_g [ log(sum_j exp(sim[g,j]) - exp(sim[g,g])) - sim[g, (g+B)%N] ]

Sharding: sim rows are sharded 512/core.  Instead of AllGather-ing the
normalized z across cores (two 64KB/core AllGathers cost ~100us+ of
per-execute collective/sync overhead each on this fabric), every core
receives the FULL z replicated — the host tiles the 2MB bf16 copy of z eight
ways, which costs H2D bandwidth once (inputs stay device-resident across
executions) and nothing per execution.  Each core also receives two 512-row
per-core shards so no on-device core-id logic is needed:

  zown = z[512c : 512c+512]            (its sim rows; just z row-sharded)
  zpos = roll(z, -2048)[512c : ...]    (positive-pair rows of those rows)

Per-core program (no collectives; Pool unused — it has no PSUM port and
its real-HW ucode speed is unverified):
  1. normalize all 4096 rows of zall in software-pipelined round groups
     ROUNDS=(2,6,8,8,8): per-tile DVE scalar_tensor_tensor sumsq (accum ops
     run at 1x so batching buys nothing), ACT exp(-0.5*ln) rsqrt, per-tile
     DVE scale; round r+1's stage is emitted before round r's compute so
     each engine's in-order queue works a round ahead
  2. PE-transpose (identity matmul) normalized tiles into znt [256, 4096]
     (PE transposes also keep the PE out of its low p-state); DVE drains
     PSUM->SBUF
  3. per column group, IMMEDIATELY after its transposes: sim matmuls
     lhsT x znt into [128 x 1024] PSUM tiles and ACT exp with accum_out ->
     per-row partial rowsums.  Own rows stay RAW: their normalization is
     the exp's per-partition scale AP (invo = 2/|z_g|); own-row sumsq runs
     on ACT (Square+accum) inside its dependency waits
  4. positives: fused DVE multiply + grouped reduce over RAW zo*zq, with
     the inv-norm correction applied to 4 scalars in the tail
  5. tail: log(rowsum - e^2) (sim[g,g] == 1/T for unit rows) -
     0.5*invo_g*invo_p*posdot, reduce free dim, partition-reduce with a
     ones-vector fp32 matmul -> single fp32 partial; host sums the 8
     partials / 4096.
"""

import numpy as np

import concourse.bacc as bacc
import concourse.bass as bass
import concourse.masks as masks
import concourse.tile as tile
from concourse import mybir

F32 = mybir.dt.float32
BF16 = mybir.dt.bfloat16
AF = mybir.ActivationFunctionType
ALU = mybir.AluOpType
AXIS = mybir.AxisListType

B = 2048
D = 256
N = 2 * B            # 4096 total rows
NCORES = 8
RPC = N // NCORES    # 512 sim rows per core
NT = N // 128        # 32 row tiles of zall
E2 = float(np.exp(np.float32(2.0)))   # exp(sim[g,g]) = exp(1/T)
NPACK = N + 2 * RPC                   # zall + zown + zpos rows, one tensor
IN_NAMES = ("zpack",)                 # single packed ExternalInput
OUT_SHAPE = (1, 1)                    # per-core output shape


class _Bacc(bacc.Bacc):
    """Bacc that pins the activation-table pass to the one set containing
    both Ln and Exp — the default fixpoint picks per-function sets and
    thrashes 5 table loads (~6.4us of ACT) into the schedule."""

    def insert_act_table_loads(self):
        from concourse.hw_specs import get_activation_tables
        import bass_rust as _bass_rust

        has_activation = any(
            isinstance(i, mybir.InstActivation)
            for b in self.main_func.blocks
            for i in b.instructions
        )
        if not has_activation:
            return
        keep = {
            mybir.ActivationFunctionType.Ln,
            mybir.ActivationFunctionType.Exp,
        }
        tables = [
            (k, v if k == "natural_log_exp_and_others" else v - keep)
            for k, v in get_activation_tables(self.m.arch).items()
        ]
        _bass_rust.insert_act_table_loads(self, tables)


def build_nc():
    nc = _Bacc("TRN2", target_bir_lowering=False, debug=False)
    zpack = nc.dram_tensor("zpack", [NPACK, D], BF16, kind="ExternalInput").ap()
    out = nc.dram_tensor("out", [1, 1], F32, kind="ExternalOutput").ap()
    with tile.TileContext(nc) as tc:
        build_tile_program(
            tc,
            out,
            zpack[0:N],
            zpack[N : N + RPC],
            zpack[N + RPC : NPACK],
        )
    nc.compile()
    return nc


# zall tile groups per pipelined round (triangular: tiny first round so the
# first exp fires early, small last round so the trailing chain is short)
ROUNDS = (2, 6, 8, 8, 8)
NROUNDS = len(ROUNDS)


def build_tile_program(
    tc: tile.TileContext, out: bass.AP, zall: bass.AP, zown: bass.AP, zpos: bass.AP
):
    nc = tc.nc
    zall_v = zall.rearrange("(t p) d -> p t d", t=NT)   # row r = t*128 + p
    zown_v = zown.rearrange("(t p) d -> p t d", t=4)
    zpos_v = zpos.rearrange("(t p) d -> p t d", t=4)

    with (
        tc.tile_pool(name="consts", bufs=1) as consts,
        tc.tile_pool(name="zp", bufs=1) as zp,
        tc.tile_pool(name="znp", bufs=1) as znp,
        tc.tile_pool(name="zntp", bufs=1) as zntp,
        tc.tile_pool(name="sqp", bufs=2) as sqp,
        tc.tile_pool(name="smalls", bufs=1) as smalls,
        tc.tile_pool(name="scrp", bufs=2) as scrp,
        tc.tile_pool(name="trps", bufs=2, space="PSUM") as trps,
        tc.tile_pool(name="simp", bufs=3, space="PSUM") as simp,
    ):
        identity = consts.tile([128, 128], BF16, tag="identity")
        masks.make_identity(nc, identity[:, :])
        zero_col = consts.tile([128, 1], F32, tag="zero_col")
        nc.vector.memset(zero_col, 0.0)
        neg_e2 = consts.tile([128, 1], F32, tag="neg_e2")
        nc.vector.memset(neg_e2, -E2)
        ones_col = consts.tile([128, 1], F32, tag="ones_col")
        nc.vector.memset(ones_col, 1.0)
        ln2_col = consts.tile([128, 1], F32, tag="ln2_col")
        nc.vector.memset(ln2_col, float(np.log(2.0)))

        # persistent tiles
        za = zp.tile([128, NT, D], BF16, tag="za")        # full z rows
        zo = zp.tile([128, 4, D], BF16, tag="zo")         # own rows (raw!)
        zq = zp.tile([128, 4, D], BF16, tag="zq")         # positive rows
        zn = znp.tile([128, NT, D], BF16, tag="zn")       # normalized full z
        pdh = znp.tile([128, 4, D], BF16, tag="pdh")      # posdot products
        znt = zntp.tile([128, 2, N], BF16, tag="znt")     # znT, all 4096 cols
        znoT = zntp.tile([128, 2, RPC], BF16, tag="znoT")  # RAW own as cols

        ssa = smalls.tile([128, NT], F32, tag="ssa")
        lna = smalls.tile([128, NT], F32, tag="lna")
        inva = smalls.tile([128, NT], F32, tag="inva")
        sso = smalls.tile([128, 8], F32, tag="sso")       # own(0:4) pos(4:8)
        lno = smalls.tile([128, 8], F32, tag="lno")
        invo = smalls.tile([128, 8], F32, tag="invo")
        acc20 = smalls.tile([128, 4 * NROUNDS], F32, tag="acc20")
        posd4 = smalls.tile([128, 4], F32, tag="posd4")
        rowsum4 = smalls.tile([128, 4], F32, tag="rowsum4")
        tailA = smalls.tile([128, 4], F32, tag="tailA")
        tot4 = smalls.tile([128, 4], F32, tag="tot4")
        tail1 = smalls.tile([128, 1], F32, tag="tail1")
        result = smalls.tile([128, 1], F32, tag="result")

        # ---- loads, all on the SP queue (DMAs triggered from nc.scalar
        # block the ACT engine for the whole transfer — never use it).
        # Round 0's two tiles ship first so its sumsq starts at ~0.7us;
        # own/pos rows next (they gate the lhsT transposes and invo);
        # then one chunk per remaining round, just-in-time.
        r0 = ROUNDS[0]
        nc.sync.dma_start(
            out=za[:, 0:r0, :], in_=zall_v[:, 0:r0, :]
        )
        nc.sync.dma_start(out=zo, in_=zown_v)
        nc.sync.dma_start(out=zq, in_=zpos_v)
        ts0 = r0
        for gsz in ROUNDS[1:]:
            nc.sync.dma_start(
                out=za[:, ts0 : ts0 + gsz, :],
                in_=zall_v[:, ts0 : ts0 + gsz, :],
            )
            ts0 += gsz

        # ---- own rows stay RAW: transpose zo into the lhsT immediately on
        # arrival (shortest possible path to the first sim matmul, and warms
        # the PE out of its low p-state), and fold the own-row normalization
        # into the exp as a per-partition scale AP: invo2 = 2/|z_g| so
        # exp(invo2_g * (z_g . zn_j)) == exp(2 * sim[g, j]).
        psT = trps.tile([128, 2, RPC], BF16, tag="tr")
        for u in range(4):
            for h in range(2):
                nc.tensor.transpose(
                    psT[:, h, u * 128 : (u + 1) * 128],
                    zo[:, u, h * 128 : (h + 1) * 128],
                    identity,
                )
        nc.vector.tensor_copy(znoT, psT)

        def own_sumsq():
            # on ACT (Square+accum): invo is only needed as the first exp's
            # scale AP (~8us in), and keeping these 8 ops off DVE lets DVE
            # pace the zall rounds ~2.4us sooner.  Emitted after round 1's
            # rsqrt so they don't delay the round pipeline's inv chain.
            for u in range(4):
                sqo = sqp.tile([128, D], BF16, tag="sqo")
                nc.scalar.activation(
                    sqo, zo[:, u, :], AF.Square,
                    bias=zero_col, accum_out=sso[:, u : u + 1],
                )
            for u in range(4):
                sqq = sqp.tile([128, D], BF16, tag="sqo")
                nc.scalar.activation(
                    sqq, zq[:, u, :], AF.Square,
                    bias=zero_col, accum_out=sso[:, 4 + u : 5 + u],
                )
            nc.scalar.activation(lno, sso, AF.Ln, bias=zero_col)
            # invo = 2/sqrt(sumsq)  (ln2 bias folds the *2 temperature)
            nc.scalar.activation(invo, lno, AF.Exp, bias=ln2_col, scale=-0.5)

        def posdots():
            # positives on RAW rows; normalization correction in the tail
            nc.vector.tensor_tensor(out=pdh, in0=zo, in1=zq, op=ALU.mult)
            nc.vector.tensor_reduce(out=posd4, in_=pdh, axis=AXIS.X, op=ALU.add)

        # ---- pipelined rounds over zall tile groups: sumsq (DVE) ->
        # rsqrt (ACT) -> scale (Pool) -> PE transpose -> drain copy
        # (Pool/DVE alternate) -> sim matmuls (PE) -> exp+rowsum (ACT).
        # Stage A of round r+1 is emitted BEFORE stage B of round r so each
        # engine's in-order queue works a round ahead (in particular Pool's
        # scales must not convoy behind its drain copies).
        bounds = []
        ts = 0
        for gsz in ROUNDS:
            bounds.append((ts, ts + gsz))
            ts += gsz

        def stage_a(r):
            ts, te = bounds[r]
            for t in range(ts, te):
                sq = sqp.tile([128, D], BF16, tag="sq")
                nc.vector.scalar_tensor_tensor(
                    out=sq, in0=za[:, t, :], scalar=1.0, in1=za[:, t, :],
                    op0=ALU.mult, op1=ALU.mult, accum_out=ssa[:, t : t + 1],
                )
            gs = slice(ts, te)
            nc.scalar.activation(lna[:, gs], ssa[:, gs], AF.Ln, bias=zero_col)
            nc.scalar.activation(
                inva[:, gs], lna[:, gs], AF.Exp, bias=zero_col, scale=-0.5
            )
            for t in range(ts, te):
                nc.vector.tensor_scalar_mul(
                    zn[:, t, :], za[:, t, :], inva[:, t : t + 1]
                )

        def stage_b(r):
            ts, te = bounds[r]
            gsz = te - ts
            cols = 128 * gsz
            c0 = 128 * ts
            for pair, t0 in enumerate(range(ts, te, 4)):
                bsz = min(4, te - t0)  # up to 4 tiles per PSUM buf
                psA = trps.tile([128, 2, RPC], BF16, tag="tr")
                for dt in range(bsz):
                    t = t0 + dt
                    for h in range(2):
                        nc.tensor.transpose(
                            psA[:, h, dt * 128 : (dt + 1) * 128],
                            zn[:, t, h * 128 : (h + 1) * 128],
                            identity,
                        )
                # PSUM reads are DVE/ACT-only on TRN2 (Pool has no PSUM port)
                nc.vector.tensor_copy(
                    znt[:, :, t0 * 128 : (t0 + bsz) * 128],
                    psA[:, :, 0 : bsz * 128],
                )
            # sim matmuls + exp (per-partition scale = 2/|z_g|) for the group
            for rc in range(4):
                ps = simp.tile([128, 1024], F32, tag="ps")
                for h in range(2):
                    lhsT = znoT[:, h, rc * 128 : (rc + 1) * 128]
                    for q in range(0, cols, 512):
                        qs = min(512, cols - q)
                        nc.tensor.matmul(
                            ps[:, q : q + qs],
                            lhsT,
                            znt[:, h, c0 + q : c0 + q + qs],
                            start=(h == 0),
                            stop=(h == 1),
                        )
                scr = scrp.tile([128, 1024], BF16, tag="exp_scr")
                k = r * 4 + rc
                nc.scalar.activation(
                    scr[:, 0:cols],
                    ps[:, 0:cols],
                    AF.Exp,
                    bias=zero_col,
                    scale=invo[:, rc : rc + 1],
                    accum_out=acc20[:, k : k + 1],
                )

        # round 0's sumsq goes FIRST on the DVE queue (it gates the first
        # exp); own-row sumsq (ACT) lands after round 1's rsqrt; posdots
        # land last (only the tail reads them).
        stage_a(0)
        for r in range(NROUNDS):
            if r + 1 < NROUNDS:
                stage_a(r + 1)
            if r == 0:
                own_sumsq()
            stage_b(r)
        posdots()

        # ---- tail: rowsums, log(neg) - sim[g,pos], partition-reduce.
        # sim[g,pos] = 2*inv_g*inv_p*posd_raw = 0.5*invo_g*invo_p*posd_raw
        acc_v = acc20.rearrange("p (s r) -> p r s", s=NROUNDS)
        nc.vector.tensor_reduce(out=rowsum4, in_=acc_v, axis=AXIS.X, op=ALU.add)
        nc.scalar.activation(tailA, rowsum4, AF.Ln, bias=neg_e2)
        ipp = smalls.tile([128, 4], F32, tag="ipp")
        nc.vector.tensor_tensor(
            out=ipp, in0=invo[:, 0:4], in1=invo[:, 4:8], op=ALU.mult
        )
        ppp = smalls.tile([128, 4], F32, tag="ppp")
        nc.vector.tensor_tensor(out=ppp, in0=ipp, in1=posd4, op=ALU.mult)
        nc.vector.scalar_tensor_tensor(
            out=tot4, in0=ppp, scalar=-0.5, in1=tailA, op0=ALU.mult, op1=ALU.add
        )
        nc.vector.tensor_reduce(out=tail1, in_=tot4, axis=AXIS.X, op=ALU.add)
        res_ps = simp.tile([128, 1024], F32, tag="ps")
        nc.tensor.matmul(
            res_ps[0:1, 0:1], ones_col, tail1, start=True, stop=True
        )
        nc.vector.tensor_copy(result[0:1, :], res_ps[0:1, 0:1])
        nc.sync.dma_start(out=out, in_=result[0:1, :])


_NC_CACHE = None


def _get_nc():
    global _NC_CACHE
    if _NC_CACHE is None:
        _NC_CACHE = build_nc()
    return _NC_CACHE


def _to_bf16(x: np.ndarray) -> np.ndarray:
    import ml_dtypes

    return np.asarray(x).astype(ml_dtypes.bfloat16)


def _host_shards(z_i: np.ndarray, z_j: np.ndarray):
    """-> (zpack_stack [8*NPACK, D],) bf16: per core, the full z followed by
    its own 512 sim rows and their 512 positive-pair rows."""
    z = np.concatenate([np.asarray(z_i), np.asarray(z_j)], axis=0)
    zbf = _to_bf16(z)
    zpos = np.roll(zbf, -B, axis=0)
    parts = []
    for c in range(NCORES):
        parts.append(zbf)
        parts.append(zbf[c * RPC : (c + 1) * RPC])
        parts.append(zpos[c * RPC : (c + 1) * RPC])
    return (np.concatenate(parts, axis=0),)


def make_in_maps(z_i: np.ndarray, z_j: np.ndarray):
    (zpack_stack,) = _host_shards(z_i, z_j)
    return [
        {"zpack": zpack_stack[c * NPACK : (c + 1) * NPACK]}
        for c in range(NCORES)
    ]


_EXEC_CACHE = None


def _get_exec():
    """Jitted 8-core SPMD executable (fast-dispatch compiled), built once."""
    global _EXEC_CACHE
    if _EXEC_CACHE is None:
        import jax
        from jax.experimental.shard_map import shard_map
        from jax.sharding import Mesh, PartitionSpec

        from concourse import bass2jax

        nc = _get_nc()
        bass2jax.install_neuronx_cc_hook()
        assert nc.dbg_addr is None
        part_name = (
            nc.partition_id_tensor.name if nc.partition_id_tensor else None
        )
        # input order: ExternalInputs, partition id.  The "out" tensor is
        # NOT passed as a donated zeroed operand (run_bass_via_pjrt does
        # that only to guarantee zero-init for kernels that don't write
        # every element — ours fully writes out[1,1]); binding it purely
        # as the custom-call result drops one operand + per-call donation
        # bookkeeping from the dispatch path.
        in_names = list(IN_NAMES) + ([part_name] if part_name else [])
        out_avals = (jax.core.ShapedArray(OUT_SHAPE, np.float32),)

        def _body(*args):
            operands = list(args)
            if part_name is not None:
                operands.append(bass2jax.partition_id_tensor())
            outs = bass2jax._bass_exec_p.bind(
                *operands,
                out_avals=out_avals,
                in_names=tuple(in_names),
                out_names=("out",),
                lowering_input_output_aliases=(),
                sim_require_finite=True,
                sim_require_nnan=True,
                nc=nc,
            )
            return tuple(outs)

        devices = jax.devices()[:NCORES]
        mesh = Mesh(np.asarray(devices), ("core",))
        n_in = len(IN_NAMES)

        def make_jit():
            return jax.jit(
                shard_map(
                    _body,
                    mesh=mesh,
                    in_specs=(PartitionSpec("core"),) * n_in,
                    out_specs=(PartitionSpec("core"),),
                    check_rep=False,
                ),
                keep_unused=True,
            )

        import ml_dtypes

        ex_args = (np.zeros((NCORES * NPACK, D), ml_dtypes.bfloat16),)
        try:
            # fast_dispatch_compile minus the FastDispatchCompiled reclass:
            # kernel() always reads its outputs, so the never-read-output
            # safety net would only add per-call host work.
            with bass2jax._fast_dispatch_active(True):
                compiled = make_jit().lower(*ex_args).compile()
            if compiled._executable.unsafe_call.has_unordered_effects:
                raise RuntimeError("bass_effect still present")
            _EXEC_CACHE = compiled
        except Exception:
            _EXEC_CACHE = make_jit()
    return _EXEC_CACHE


def kernel(z_i: np.ndarray, z_j: np.ndarray) -> np.ndarray:
    """Full inputs in, full output out; shards sim rows across the 8 cores."""
    sharded = _get_exec()
    (zpack_stack,) = _host_shards(z_i, z_j)
    (partials,) = sharded(zpack_stack)
    return np.float32(float(np.asarray(partials).sum()) / N)


# revision 53
# speedup vs baseline: 1.2328x; 1.2122x over previous
"""SimCLR NT-Xent contrastive loss on 8 Trainium2 cores — collective-free.

Reference math (B=2048, D=256, T=0.5):
    zn = l2norm_rows(concat(z_i, z_j))          # [4096, 256]
    sim = zn @ zn.T / T                         # [4096, 4096]
    loss = mean_g [ log(sum_j exp(sim[g,j]) - exp(sim[g,g])) - sim[g, (g+B)%N] ]

Sharding: sim rows are sharded 512/core.  Instead of AllGather-ing the
normalized z across cores (two 64KB/core AllGathers cost ~100us+ of
per-execute collective/sync overhead each on this fabric), every core
receives the FULL z replicated — the host tiles the 2MB bf16 copy of z eight
ways, which costs H2D bandwidth once (inputs stay device-resident across
executions) and nothing per execution.  Each core also receives two 512-row
per-core shards so no on-device core-id logic is needed:

  zown = z[512c : 512c+512]            (its sim rows; just z row-sharded)
  zpos = roll(z, -2048)[512c : ...]    (positive-pair rows of those rows)

Per-core program (no collectives; Pool unused — it has no PSUM port and
its real-HW ucode speed is unverified):
  1. normalize all 4096 rows of zall in software-pipelined round groups
     ROUNDS=(2,6,8,8,8): per-tile DVE scalar_tensor_tensor sumsq (accum ops
     run at 1x so batching buys nothing), ACT exp(-0.5*ln) rsqrt, per-tile
     DVE scale; round r+1's stage is emitted before round r's compute so
     each engine's in-order queue works a round ahead
  2. PE-transpose (identity matmul) normalized tiles into znt [256, 4096]
     (PE transposes also keep the PE out of its low p-state); DVE drains
     PSUM->SBUF
  3. per column group, IMMEDIATELY after its transposes: sim matmuls
     lhsT x znt into [128 x 1024] PSUM tiles and ACT exp with accum_out ->
     per-row partial rowsums.  Own rows stay RAW: their normalization is
     the exp's per-partition scale AP (invo = 2/|z_g|); own-row sumsq runs
     on ACT (Square+accum) inside its dependency waits
  4. positives: fused DVE multiply + grouped reduce over RAW zo*zq, with
     the inv-norm correction applied to 4 scalars in the tail
  5. tail: log(rowsum - e^2) (sim[g,g] == 1/T for unit rows) -
     0.5*invo_g*invo_p*posdot, reduce free dim, partition-reduce with a
     ones-vector fp32 matmul -> single fp32 partial; host sums the 8
     partials / 4096.
"""

import numpy as np

import concourse.bacc as bacc
import concourse.bass as bass
import concourse.masks as masks
import concourse.tile as tile
from concourse import mybir

F32 = mybir.dt.float32
BF16 = mybir.dt.bfloat16
AF = mybir.ActivationFunctionType
ALU = mybir.AluOpType
AXIS = mybir.AxisListType

B = 2048
D = 256
N = 2 * B            # 4096 total rows
NCORES = 8
RPC = N // NCORES    # 512 sim rows per core
NT = N // 128        # 32 row tiles of zall
E2 = float(np.exp(np.float32(2.0)))   # exp(sim[g,g]) = exp(1/T)
NPACK = N + 2 * RPC                   # zall + zown + zpos rows, one tensor
IN_NAMES = ("zpack",)                 # single packed ExternalInput
OUT_SHAPE = (1, 1)                    # per-core output shape


class _Bacc(bacc.Bacc):
    """Bacc that pins the activation-table pass to the one set containing
    both Ln and Exp — the default fixpoint picks per-function sets and
    thrashes 5 table loads (~6.4us of ACT) into the schedule."""

    def insert_act_table_loads(self):
        from concourse.hw_specs import get_activation_tables
        import bass_rust as _bass_rust

        has_activation = any(
            isinstance(i, mybir.InstActivation)
            for b in self.main_func.blocks
            for i in b.instructions
        )
        if not has_activation:
            return
        keep = {
            mybir.ActivationFunctionType.Ln,
            mybir.ActivationFunctionType.Exp,
        }
        tables = [
            (k, v if k == "natural_log_exp_and_others" else v - keep)
            for k, v in get_activation_tables(self.m.arch).items()
        ]
        _bass_rust.insert_act_table_loads(self, tables)


def build_nc():
    nc = _Bacc("TRN2", target_bir_lowering=False, debug=False)
    zpack = nc.dram_tensor("zpack", [NPACK, D], BF16, kind="ExternalInput").ap()
    out = nc.dram_tensor("out", [1, 1], F32, kind="ExternalOutput").ap()
    with tile.TileContext(nc) as tc:
        build_tile_program(
            tc,
            out,
            zpack[0:N],
            zpack[N : N + RPC],
            zpack[N + RPC : NPACK],
        )
    nc.compile()
    return nc


# zall tile groups per pipelined round (triangular: tiny first round so the
# first exp fires early, small last round so the trailing chain is short)
ROUNDS = (2, 6, 8, 8, 8)
NROUNDS = len(ROUNDS)


def build_tile_program(
    tc: tile.TileContext, out: bass.AP, zall: bass.AP, zown: bass.AP, zpos: bass.AP
):
    nc = tc.nc
    zall_v = zall.rearrange("(t p) d -> p t d", t=NT)   # row r = t*128 + p
    zown_v = zown.rearrange("(t p) d -> p t d", t=4)
    zpos_v = zpos.rearrange("(t p) d -> p t d", t=4)

    with (
        tc.tile_pool(name="consts", bufs=1) as consts,
        tc.tile_pool(name="zp", bufs=1) as zp,
        tc.tile_pool(name="znp", bufs=1) as znp,
        tc.tile_pool(name="zntp", bufs=1) as zntp,
        tc.tile_pool(name="sqp", bufs=2) as sqp,
        tc.tile_pool(name="smalls", bufs=1) as smalls,
        tc.tile_pool(name="scrp", bufs=2) as scrp,
        tc.tile_pool(name="trps", bufs=2, space="PSUM") as trps,
        tc.tile_pool(name="simp", bufs=3, space="PSUM") as simp,
    ):
        identity = consts.tile([128, 128], BF16, tag="identity")
        masks.make_identity(nc, identity[:, :])
        zero_col = consts.tile([128, 1], F32, tag="zero_col")
        nc.vector.memset(zero_col, 0.0)
        neg_e2 = consts.tile([128, 1], F32, tag="neg_e2")
        nc.vector.memset(neg_e2, -E2)
        ones_col = consts.tile([128, 1], F32, tag="ones_col")
        nc.vector.memset(ones_col, 1.0)
        ln2_col = consts.tile([128, 1], F32, tag="ln2_col")
        nc.vector.memset(ln2_col, float(np.log(2.0)))

        # persistent tiles
        za = zp.tile([128, NT, D], BF16, tag="za")        # full z rows
        zo = zp.tile([128, 4, D], BF16, tag="zo")         # own rows (raw!)
        zq = zp.tile([128, 4, D], BF16, tag="zq")         # positive rows
        zn = znp.tile([128, NT, D], BF16, tag="zn")       # normalized full z
        pdh = znp.tile([128, 4, D], BF16, tag="pdh")      # posdot products
        znt = zntp.tile([128, 2, N], BF16, tag="znt")     # znT, all 4096 cols
        znoT = zntp.tile([128, 2, RPC], BF16, tag="znoT")  # RAW own as cols

        ssa = smalls.tile([128, NT], F32, tag="ssa")
        lna = smalls.tile([128, NT], F32, tag="lna")
        inva = smalls.tile([128, NT], F32, tag="inva")
        sso = smalls.tile([128, 8], F32, tag="sso")       # own(0:4) pos(4:8)
        lno = smalls.tile([128, 8], F32, tag="lno")
        invo = smalls.tile([128, 8], F32, tag="invo")
        acc20 = smalls.tile([128, 4 * NROUNDS], F32, tag="acc20")
        posd4 = smalls.tile([128, 4], F32, tag="posd4")
        rowsum4 = smalls.tile([128, 4], F32, tag="rowsum4")
        tailA = smalls.tile([128, 4], F32, tag="tailA")
        tot4 = smalls.tile([128, 4], F32, tag="tot4")
        tail1 = smalls.tile([128, 1], F32, tag="tail1")
        result = smalls.tile([128, 1], F32, tag="result")

        # ---- loads, all on the SP queue (DMAs triggered from nc.scalar
        # block the ACT engine for the whole transfer — never use it).
        # Round 0's two tiles ship first so its sumsq starts at ~0.7us;
        # own/pos rows next (they gate the lhsT transposes and invo);
        # then one chunk per remaining round, just-in-time.
        r0 = ROUNDS[0]
        nc.sync.dma_start(
            out=za[:, 0:r0, :], in_=zall_v[:, 0:r0, :]
        )
        nc.sync.dma_start(out=zo, in_=zown_v)
        nc.sync.dma_start(out=zq, in_=zpos_v)
        ts0 = r0
        for gsz in ROUNDS[1:]:
            nc.sync.dma_start(
                out=za[:, ts0 : ts0 + gsz, :],
                in_=zall_v[:, ts0 : ts0 + gsz, :],
            )
            ts0 += gsz

        # ---- own rows stay RAW: transpose zo into the lhsT immediately on
        # arrival (shortest possible path to the first sim matmul, and warms
        # the PE out of its low p-state), and fold the own-row normalization
        # into the exp as a per-partition scale AP: invo2 = 2/|z_g| so
        # exp(invo2_g * (z_g . zn_j)) == exp(2 * sim[g, j]).
        psT = trps.tile([128, 2, RPC], BF16, tag="tr")
        for u in range(4):
            for h in range(2):
                nc.tensor.transpose(
                    psT[:, h, u * 128 : (u + 1) * 128],
                    zo[:, u, h * 128 : (h + 1) * 128],
                    identity,
                )
        nc.vector.tensor_copy(znoT, psT)

        def own_sumsq():
            # on ACT (Square+accum): invo is only needed as the first exp's
            # scale AP (~8us in), and keeping these 8 ops off DVE lets DVE
            # pace the zall rounds ~2.4us sooner.  Emitted after round 1's
            # rsqrt so they don't delay the round pipeline's inv chain.
            for u in range(4):
                sqo = sqp.tile([128, D], BF16, tag="sqo")
                nc.scalar.activation(
                    sqo, zo[:, u, :], AF.Square,
                    bias=zero_col, accum_out=sso[:, u : u + 1],
                )
            for u in range(4):
                sqq = sqp.tile([128, D], BF16, tag="sqo")
                nc.scalar.activation(
                    sqq, zq[:, u, :], AF.Square,
                    bias=zero_col, accum_out=sso[:, 4 + u : 5 + u],
                )
            nc.scalar.activation(lno, sso, AF.Ln, bias=zero_col)
            # invo = 2/sqrt(sumsq)  (ln2 bias folds the *2 temperature)
            nc.scalar.activation(invo, lno, AF.Exp, bias=ln2_col, scale=-0.5)

        def posdots():
            # positives on RAW rows; normalization correction in the tail
            nc.vector.tensor_tensor(out=pdh, in0=zo, in1=zq, op=ALU.mult)
            nc.vector.tensor_reduce(out=posd4, in_=pdh, axis=AXIS.X, op=ALU.add)

        # ---- pipelined rounds over zall tile groups: sumsq (DVE) ->
        # rsqrt (ACT) -> scale (Pool) -> PE transpose -> drain copy
        # (Pool/DVE alternate) -> sim matmuls (PE) -> exp+rowsum (ACT).
        # Stage A of round r+1 is emitted BEFORE stage B of round r so each
        # engine's in-order queue works a round ahead (in particular Pool's
        # scales must not convoy behind its drain copies).
        bounds = []
        ts = 0
        for gsz in ROUNDS:
            bounds.append((ts, ts + gsz))
            ts += gsz

        def stage_a(r):
            ts, te = bounds[r]
            for t in range(ts, te):
                sq = sqp.tile([128, D], BF16, tag="sq")
                nc.vector.scalar_tensor_tensor(
                    out=sq, in0=za[:, t, :], scalar=1.0, in1=za[:, t, :],
                    op0=ALU.mult, op1=ALU.mult, accum_out=ssa[:, t : t + 1],
                )
            gs = slice(ts, te)
            nc.scalar.activation(lna[:, gs], ssa[:, gs], AF.Ln, bias=zero_col)
            nc.scalar.activation(
                inva[:, gs], lna[:, gs], AF.Exp, bias=zero_col, scale=-0.5
            )
            for t in range(ts, te):
                nc.vector.tensor_scalar_mul(
                    zn[:, t, :], za[:, t, :], inva[:, t : t + 1]
                )

        def stage_b(r):
            ts, te = bounds[r]
            gsz = te - ts
            cols = 128 * gsz
            c0 = 128 * ts
            for pair, t0 in enumerate(range(ts, te, 4)):
                bsz = min(4, te - t0)  # up to 4 tiles per PSUM buf
                psA = trps.tile([128, 2, RPC], BF16, tag="tr")
                for dt in range(bsz):
                    t = t0 + dt
                    for h in range(2):
                        nc.tensor.transpose(
                            psA[:, h, dt * 128 : (dt + 1) * 128],
                            zn[:, t, h * 128 : (h + 1) * 128],
                            identity,
                        )
                # PSUM reads are DVE/ACT-only on TRN2 (Pool has no PSUM port)
                nc.vector.tensor_copy(
                    znt[:, :, t0 * 128 : (t0 + bsz) * 128],
                    psA[:, :, 0 : bsz * 128],
                )
            # sim matmuls + exp (per-partition scale = 2/|z_g|) for the group
            for rc in range(4):
                ps = simp.tile([128, 1024], F32, tag="ps")
                for h in range(2):
                    lhsT = znoT[:, h, rc * 128 : (rc + 1) * 128]
                    for q in range(0, cols, 512):
                        qs = min(512, cols - q)
                        nc.tensor.matmul(
                            ps[:, q : q + qs],
                            lhsT,
                            znt[:, h, c0 + q : c0 + q + qs],
                            start=(h == 0),
                            stop=(h == 1),
                        )
                scr = scrp.tile([128, 1024], BF16, tag="exp_scr")
                k = r * 4 + rc
                nc.scalar.activation(
                    scr[:, 0:cols],
                    ps[:, 0:cols],
                    AF.Exp,
                    bias=zero_col,
                    scale=invo[:, rc : rc + 1],
                    accum_out=acc20[:, k : k + 1],
                )

        # round 0's sumsq goes FIRST on the DVE queue (it gates the first
        # exp); own-row sumsq (ACT) lands after round 1's rsqrt; posdots
        # land last (only the tail reads them).
        stage_a(0)
        for r in range(NROUNDS):
            if r + 1 < NROUNDS:
                stage_a(r + 1)
            if r == 0:
                own_sumsq()
            stage_b(r)
        posdots()

        # ---- tail: rowsums, log(neg) - sim[g,pos], partition-reduce.
        # sim[g,pos] = 2*inv_g*inv_p*posd_raw = 0.5*invo_g*invo_p*posd_raw
        acc_v = acc20.rearrange("p (s r) -> p r s", s=NROUNDS)
        nc.vector.tensor_reduce(out=rowsum4, in_=acc_v, axis=AXIS.X, op=ALU.add)
        nc.scalar.activation(tailA, rowsum4, AF.Ln, bias=neg_e2)
        ipp = smalls.tile([128, 4], F32, tag="ipp")
        nc.vector.tensor_tensor(
            out=ipp, in0=invo[:, 0:4], in1=invo[:, 4:8], op=ALU.mult
        )
        ppp = smalls.tile([128, 4], F32, tag="ppp")
        nc.vector.tensor_tensor(out=ppp, in0=ipp, in1=posd4, op=ALU.mult)
        nc.vector.scalar_tensor_tensor(
            out=tot4, in0=ppp, scalar=-0.5, in1=tailA, op0=ALU.mult, op1=ALU.add
        )
        nc.vector.tensor_reduce(out=tail1, in_=tot4, axis=AXIS.X, op=ALU.add)
        res_ps = simp.tile([128, 1024], F32, tag="ps")
        nc.tensor.matmul(
            res_ps[0:1, 0:1], ones_col, tail1, start=True, stop=True
        )
        nc.vector.tensor_copy(result[0:1, :], res_ps[0:1, 0:1])
        nc.sync.dma_start(out=out, in_=result[0:1, :])


_NC_CACHE = None


def _get_nc():
    global _NC_CACHE
    if _NC_CACHE is None:
        _NC_CACHE = build_nc()
    return _NC_CACHE


def _to_bf16(x: np.ndarray) -> np.ndarray:
    import ml_dtypes

    return np.asarray(x).astype(ml_dtypes.bfloat16)


def _host_shards(z_i: np.ndarray, z_j: np.ndarray):
    """-> (zpack_stack [8*NPACK, D],) bf16: per core, the full z followed by
    its own 512 sim rows and their 512 positive-pair rows."""
    z = np.concatenate([np.asarray(z_i), np.asarray(z_j)], axis=0)
    zbf = _to_bf16(z)
    zpos = np.roll(zbf, -B, axis=0)
    parts = []
    for c in range(NCORES):
        parts.append(zbf)
        parts.append(zbf[c * RPC : (c + 1) * RPC])
        parts.append(zpos[c * RPC : (c + 1) * RPC])
    return (np.concatenate(parts, axis=0),)


def make_in_maps(z_i: np.ndarray, z_j: np.ndarray):
    (zpack_stack,) = _host_shards(z_i, z_j)
    return [
        {"zpack": zpack_stack[c * NPACK : (c + 1) * NPACK]}
        for c in range(NCORES)
    ]


_EXEC_CACHE = None


def _get_exec():
    """Jitted 8-core SPMD executable (fast-dispatch compiled), built once."""
    global _EXEC_CACHE
    if _EXEC_CACHE is None:
        import jax
        from jax.experimental.shard_map import shard_map
        from jax.sharding import Mesh, PartitionSpec

        from concourse import bass2jax

        nc = _get_nc()
        bass2jax.install_neuronx_cc_hook()
        assert nc.dbg_addr is None
        part_name = (
            nc.partition_id_tensor.name if nc.partition_id_tensor else None
        )
        # input order: ExternalInputs, partition id.  The "out" tensor is
        # NOT passed as a donated zeroed operand (run_bass_via_pjrt does
        # that only to guarantee zero-init for kernels that don't write
        # every element — ours fully writes out[1,1]); binding it purely
        # as the custom-call result drops one operand + per-call donation
        # bookkeeping from the dispatch path.
        in_names = list(IN_NAMES) + ([part_name] if part_name else [])
        out_avals = (jax.core.ShapedArray(OUT_SHAPE, np.float32),)

        def _body(*args):
            operands = list(args)
            if part_name is not None:
                operands.append(bass2jax.partition_id_tensor())
            outs = bass2jax._bass_exec_p.bind(
                *operands,
                out_avals=out_avals,
                in_names=tuple(in_names),
                out_names=("out",),
                lowering_input_output_aliases=(),
                sim_require_finite=True,
                sim_require_nnan=True,
                nc=nc,
            )
            return tuple(outs)

        devices = jax.devices()[:NCORES]
        mesh = Mesh(np.asarray(devices), ("core",))
        n_in = len(IN_NAMES)

        def make_jit():
            return jax.jit(
                shard_map(
                    _body,
                    mesh=mesh,
                    in_specs=(PartitionSpec("core"),) * n_in,
                    out_specs=(PartitionSpec("core"),),
                    check_rep=False,
                ),
                keep_unused=True,
            )

        import ml_dtypes

        ex_args = (np.zeros((NCORES * NPACK, D), ml_dtypes.bfloat16),)
        try:
            # fast_dispatch_compile minus the FastDispatchCompiled reclass:
            # kernel() always reads its outputs, so the never-read-output
            # safety net would only add per-call host work.
            with bass2jax._fast_dispatch_active(True):
                compiled = make_jit().lower(*ex_args).compile()
            if compiled._executable.unsafe_call.has_unordered_effects:
                raise RuntimeError("bass_effect still present")
            _EXEC_CACHE = compiled
        except Exception:
            _EXEC_CACHE = make_jit()
    return _EXEC_CACHE


def kernel(z_i: np.ndarray, z_j: np.ndarray) -> np.ndarray:
    """Full inputs in, full output out; shards sim rows across the 8 cores."""
    sharded = _get_exec()
    (zpack_stack,) = _host_shards(z_i, z_j)
    (partials,) = sharded(zpack_stack)
    return np.float32(float(np.asarray(partials).sum()) / N)


# revision 57
# speedup vs baseline: 1.4157x; 1.1484x over previous
"""SimCLR NT-Xent contrastive loss on 8 Trainium2 cores — collective-free.

Reference math (B=2048, D=256, T=0.5):
    zn = l2norm_rows(concat(z_i, z_j))          # [4096, 256]
    sim = zn @ zn.T / T                         # [4096, 4096]
    loss = mean_g [ log(sum_j exp(sim[g,j]) - exp(sim[g,g])) - sim[g, (g+B)%N] ]

Sharding: sim rows are sharded 512/core.  Instead of AllGather-ing the
normalized z across cores (two 64KB/core AllGathers cost ~100us+ of
per-execute collective/sync overhead each on this fabric), every core
receives the FULL z replicated — the host tiles the 2MB bf16 copy of z eight
ways, which costs H2D bandwidth once (inputs stay device-resident across
executions) and nothing per execution.  Each core also receives two 512-row
per-core shards so no on-device core-id logic is needed:

  zown = z[512c : 512c+512]            (its sim rows; just z row-sharded)
  zpos = roll(z, -2048)[512c : ...]    (positive-pair rows of those rows)

Per-core program (no collectives; Pool unused — it has no PSUM port and
its real-HW ucode speed is unverified):
  1. normalize all 4096 rows of zall in software-pipelined round groups
     ROUNDS=(2,6,8,8,8): per-tile DVE scalar_tensor_tensor sumsq (accum ops
     run at 1x so batching buys nothing), ACT exp(-0.5*ln) rsqrt, per-tile
     DVE scale; round r+1's stage is emitted before round r's compute so
     each engine's in-order queue works a round ahead
  2. PE-transpose (identity matmul) normalized tiles into znt [256, 4096]
     (PE transposes also keep the PE out of its low p-state); DVE drains
     PSUM->SBUF
  3. per column group, IMMEDIATELY after its transposes: sim matmuls
     lhsT x znt into [128 x 1024] PSUM tiles and ACT exp with accum_out ->
     per-row partial rowsums.  Own rows stay RAW: their normalization is
     the exp's per-partition scale AP (invo = 2/|z_g|); own-row sumsq runs
     on ACT (Square+accum) inside its dependency waits
  4. positives: fused DVE multiply + grouped reduce over RAW zo*zq, with
     the inv-norm correction applied to 4 scalars in the tail
  5. tail: log(rowsum - e^2) (sim[g,g] == 1/T for unit rows) -
     0.5*invo_g*invo_p*posdot, reduce free dim, partition-reduce with a
     ones-vector fp32 matmul -> single fp32 partial; host sums the 8
     partials / 4096.
"""

import numpy as np

import concourse.bacc as bacc
import concourse.bass as bass
import concourse.masks as masks
import concourse.tile as tile
from concourse import mybir

F32 = mybir.dt.float32
BF16 = mybir.dt.bfloat16
AF = mybir.ActivationFunctionType
ALU = mybir.AluOpType
AXIS = mybir.AxisListType

B = 2048
D = 256
N = 2 * B            # 4096 total rows
NCORES = 8
RPC = N // NCORES    # 512 sim rows per core
NT = N // 128        # 32 row tiles of zall
E2 = float(np.exp(np.float32(2.0)))   # exp(sim[g,g]) = exp(1/T)
NPACK = N + 2 * RPC                   # zall + zown + zpos rows, one tensor
IN_NAMES = ("zpack",)                 # single packed ExternalInput
OUT_SHAPE = (1, 1)                    # per-core output shape


class _Bacc(bacc.Bacc):
    """Bacc that pins the activation-table pass to the one set containing
    both Ln and Exp — the default fixpoint picks per-function sets and
    thrashes 5 table loads (~6.4us of ACT) into the schedule."""

    def insert_act_table_loads(self):
        from concourse.hw_specs import get_activation_tables
        import bass_rust as _bass_rust

        has_activation = any(
            isinstance(i, mybir.InstActivation)
            for b in self.main_func.blocks
            for i in b.instructions
        )
        if not has_activation:
            return
        keep = {
            mybir.ActivationFunctionType.Ln,
            mybir.ActivationFunctionType.Exp,
        }
        tables = [
            (k, v if k == "natural_log_exp_and_others" else v - keep)
            for k, v in get_activation_tables(self.m.arch).items()
        ]
        _bass_rust.insert_act_table_loads(self, tables)


def build_nc():
    nc = _Bacc("TRN2", target_bir_lowering=False, debug=False)
    zpack = nc.dram_tensor("zpack", [NPACK, D], BF16, kind="ExternalInput").ap()
    out = nc.dram_tensor("out", [1, 1], F32, kind="ExternalOutput").ap()
    with tile.TileContext(nc) as tc:
        build_tile_program(
            tc,
            out,
            zpack[0:N],
            zpack[N : N + RPC],
            zpack[N + RPC : NPACK],
        )
    nc.compile()
    return nc


# zall tile groups per pipelined round (triangular: tiny first round so the
# first exp fires early, small last round so the trailing chain is short)
ROUNDS = (2, 6, 8, 8, 8)
NROUNDS = len(ROUNDS)


def build_tile_program(
    tc: tile.TileContext, out: bass.AP, zall: bass.AP, zown: bass.AP, zpos: bass.AP
):
    nc = tc.nc
    zall_v = zall.rearrange("(t p) d -> p t d", t=NT)   # row r = t*128 + p
    zown_v = zown.rearrange("(t p) d -> p t d", t=4)
    zpos_v = zpos.rearrange("(t p) d -> p t d", t=4)

    with (
        tc.tile_pool(name="consts", bufs=1) as consts,
        tc.tile_pool(name="zp", bufs=1) as zp,
        tc.tile_pool(name="znp", bufs=1) as znp,
        tc.tile_pool(name="zntp", bufs=1) as zntp,
        tc.tile_pool(name="sqp", bufs=2) as sqp,
        tc.tile_pool(name="smalls", bufs=1) as smalls,
        tc.tile_pool(name="scrp", bufs=2) as scrp,
        tc.tile_pool(name="trps", bufs=2, space="PSUM") as trps,
        tc.tile_pool(name="simp", bufs=3, space="PSUM") as simp,
    ):
        identity = consts.tile([128, 128], BF16, tag="identity")
        masks.make_identity(nc, identity[:, :])
        zero_col = consts.tile([128, 1], F32, tag="zero_col")
        nc.vector.memset(zero_col, 0.0)
        neg_e2 = consts.tile([128, 1], F32, tag="neg_e2")
        nc.vector.memset(neg_e2, -E2)
        ones_col = consts.tile([128, 1], F32, tag="ones_col")
        nc.vector.memset(ones_col, 1.0)
        ln2_col = consts.tile([128, 1], F32, tag="ln2_col")
        nc.vector.memset(ln2_col, float(np.log(2.0)))

        # persistent tiles
        za = zp.tile([128, NT, D], BF16, tag="za")        # full z rows
        zo = zp.tile([128, 4, D], BF16, tag="zo")         # own rows (raw!)
        zq = zp.tile([128, 4, D], BF16, tag="zq")         # positive rows
        zn = znp.tile([128, NT, D], BF16, tag="zn")       # normalized full z
        pdh = znp.tile([128, 4, D], BF16, tag="pdh")      # posdot products
        znt = zntp.tile([128, 2, N], BF16, tag="znt")     # znT, all 4096 cols
        znoT = zntp.tile([128, 2, RPC], BF16, tag="znoT")  # RAW own as cols

        ssa = smalls.tile([128, NT], F32, tag="ssa")
        lna = smalls.tile([128, NT], F32, tag="lna")
        inva = smalls.tile([128, NT], F32, tag="inva")
        sso = smalls.tile([128, 8], F32, tag="sso")       # own(0:4) pos(4:8)
        lno = smalls.tile([128, 8], F32, tag="lno")
        invo = smalls.tile([128, 8], F32, tag="invo")
        acc20 = smalls.tile([128, 4 * NROUNDS], F32, tag="acc20")
        posd4 = smalls.tile([128, 4], F32, tag="posd4")
        rowsum4 = smalls.tile([128, 4], F32, tag="rowsum4")
        tailA = smalls.tile([128, 4], F32, tag="tailA")
        tot4 = smalls.tile([128, 4], F32, tag="tot4")
        tail1 = smalls.tile([128, 1], F32, tag="tail1")
        result = smalls.tile([128, 1], F32, tag="result")

        # ---- loads, all on the SP queue (DMAs triggered from nc.scalar
        # block the ACT engine for the whole transfer — never use it).
        # Round 0's two tiles ship first so its sumsq starts at ~0.7us;
        # own/pos rows next (they gate the lhsT transposes and invo);
        # then one chunk per remaining round, just-in-time.
        r0 = ROUNDS[0]
        nc.sync.dma_start(
            out=za[:, 0:r0, :], in_=zall_v[:, 0:r0, :]
        )
        nc.sync.dma_start(out=zo, in_=zown_v)
        nc.sync.dma_start(out=zq, in_=zpos_v)
        ts0 = r0
        for gsz in ROUNDS[1:]:
            nc.sync.dma_start(
                out=za[:, ts0 : ts0 + gsz, :],
                in_=zall_v[:, ts0 : ts0 + gsz, :],
            )
            ts0 += gsz

        # ---- own rows stay RAW: transpose zo into the lhsT immediately on
        # arrival (shortest possible path to the first sim matmul, and warms
        # the PE out of its low p-state), and fold the own-row normalization
        # into the exp as a per-partition scale AP: invo2 = 2/|z_g| so
        # exp(invo2_g * (z_g . zn_j)) == exp(2 * sim[g, j]).
        psT = trps.tile([128, 2, RPC], BF16, tag="tr")
        for u in range(4):
            for h in range(2):
                nc.tensor.transpose(
                    psT[:, h, u * 128 : (u + 1) * 128],
                    zo[:, u, h * 128 : (h + 1) * 128],
                    identity,
                )
        nc.vector.tensor_copy(znoT, psT)

        def own_sumsq():
            # on ACT (Square+accum): invo is only needed as the first exp's
            # scale AP (~8us in), and keeping these 8 ops off DVE lets DVE
            # pace the zall rounds ~2.4us sooner.  Emitted after round 1's
            # rsqrt so they don't delay the round pipeline's inv chain.
            for u in range(4):
                sqo = sqp.tile([128, D], BF16, tag="sqo")
                nc.scalar.activation(
                    sqo, zo[:, u, :], AF.Square,
                    bias=zero_col, accum_out=sso[:, u : u + 1],
                )
            for u in range(4):
                sqq = sqp.tile([128, D], BF16, tag="sqo")
                nc.scalar.activation(
                    sqq, zq[:, u, :], AF.Square,
                    bias=zero_col, accum_out=sso[:, 4 + u : 5 + u],
                )
            nc.scalar.activation(lno, sso, AF.Ln, bias=zero_col)
            # invo = 2/sqrt(sumsq)  (ln2 bias folds the *2 temperature)
            nc.scalar.activation(invo, lno, AF.Exp, bias=ln2_col, scale=-0.5)

        def posdots():
            # positives on RAW rows; normalization correction in the tail
            nc.vector.tensor_tensor(out=pdh, in0=zo, in1=zq, op=ALU.mult)
            nc.vector.tensor_reduce(out=posd4, in_=pdh, axis=AXIS.X, op=ALU.add)

        # ---- pipelined rounds over zall tile groups: sumsq (DVE) ->
        # rsqrt (ACT) -> scale (Pool) -> PE transpose -> drain copy
        # (Pool/DVE alternate) -> sim matmuls (PE) -> exp+rowsum (ACT).
        # Stage A of round r+1 is emitted BEFORE stage B of round r so each
        # engine's in-order queue works a round ahead (in particular Pool's
        # scales must not convoy behind its drain copies).
        bounds = []
        ts = 0
        for gsz in ROUNDS:
            bounds.append((ts, ts + gsz))
            ts += gsz

        def stage_a(r):
            ts, te = bounds[r]
            for t in range(ts, te):
                sq = sqp.tile([128, D], BF16, tag="sq")
                nc.vector.scalar_tensor_tensor(
                    out=sq, in0=za[:, t, :], scalar=1.0, in1=za[:, t, :],
                    op0=ALU.mult, op1=ALU.mult, accum_out=ssa[:, t : t + 1],
                )
            gs = slice(ts, te)
            nc.scalar.activation(lna[:, gs], ssa[:, gs], AF.Ln, bias=zero_col)
            nc.scalar.activation(
                inva[:, gs], lna[:, gs], AF.Exp, bias=zero_col, scale=-0.5
            )
            for t in range(ts, te):
                nc.vector.tensor_scalar_mul(
                    zn[:, t, :], za[:, t, :], inva[:, t : t + 1]
                )

        def stage_b(r):
            ts, te = bounds[r]
            gsz = te - ts
            cols = 128 * gsz
            c0 = 128 * ts
            for pair, t0 in enumerate(range(ts, te, 4)):
                bsz = min(4, te - t0)  # up to 4 tiles per PSUM buf
                psA = trps.tile([128, 2, RPC], BF16, tag="tr")
                for dt in range(bsz):
                    t = t0 + dt
                    for h in range(2):
                        nc.tensor.transpose(
                            psA[:, h, dt * 128 : (dt + 1) * 128],
                            zn[:, t, h * 128 : (h + 1) * 128],
                            identity,
                        )
                # PSUM reads are DVE/ACT-only on TRN2 (Pool has no PSUM port)
                nc.vector.tensor_copy(
                    znt[:, :, t0 * 128 : (t0 + bsz) * 128],
                    psA[:, :, 0 : bsz * 128],
                )
            # sim matmuls + exp (per-partition scale = 2/|z_g|) for the group
            for rc in range(4):
                ps = simp.tile([128, 1024], F32, tag="ps")
                for h in range(2):
                    lhsT = znoT[:, h, rc * 128 : (rc + 1) * 128]
                    for q in range(0, cols, 512):
                        qs = min(512, cols - q)
                        nc.tensor.matmul(
                            ps[:, q : q + qs],
                            lhsT,
                            znt[:, h, c0 + q : c0 + q + qs],
                            start=(h == 0),
                            stop=(h == 1),
                        )
                scr = scrp.tile([128, 1024], BF16, tag="exp_scr")
                k = r * 4 + rc
                nc.scalar.activation(
                    scr[:, 0:cols],
                    ps[:, 0:cols],
                    AF.Exp,
                    bias=zero_col,
                    scale=invo[:, rc : rc + 1],
                    accum_out=acc20[:, k : k + 1],
                )

        # round 0's sumsq goes FIRST on the DVE queue (it gates the first
        # exp); own-row sumsq (ACT) lands after round 1's rsqrt; posdots
        # land last (only the tail reads them).
        stage_a(0)
        for r in range(NROUNDS):
            if r + 1 < NROUNDS:
                stage_a(r + 1)
            if r == 0:
                own_sumsq()
            stage_b(r)
        posdots()

        # ---- tail: rowsums, log(neg) - sim[g,pos], partition-reduce.
        # sim[g,pos] = 2*inv_g*inv_p*posd_raw = 0.5*invo_g*invo_p*posd_raw
        acc_v = acc20.rearrange("p (s r) -> p r s", s=NROUNDS)
        nc.vector.tensor_reduce(out=rowsum4, in_=acc_v, axis=AXIS.X, op=ALU.add)
        nc.scalar.activation(tailA, rowsum4, AF.Ln, bias=neg_e2)
        ipp = smalls.tile([128, 4], F32, tag="ipp")
        nc.vector.tensor_tensor(
            out=ipp, in0=invo[:, 0:4], in1=invo[:, 4:8], op=ALU.mult
        )
        ppp = smalls.tile([128, 4], F32, tag="ppp")
        nc.vector.tensor_tensor(out=ppp, in0=ipp, in1=posd4, op=ALU.mult)
        nc.vector.scalar_tensor_tensor(
            out=tot4, in0=ppp, scalar=-0.5, in1=tailA, op0=ALU.mult, op1=ALU.add
        )
        nc.vector.tensor_reduce(out=tail1, in_=tot4, axis=AXIS.X, op=ALU.add)
        res_ps = simp.tile([128, 1024], F32, tag="ps")
        nc.tensor.matmul(
            res_ps[0:1, 0:1], ones_col, tail1, start=True, stop=True
        )
        nc.vector.tensor_copy(result[0:1, :], res_ps[0:1, 0:1])
        nc.sync.dma_start(out=out, in_=result[0:1, :])


_NC_CACHE = None


def _get_nc():
    global _NC_CACHE
    if _NC_CACHE is None:
        _NC_CACHE = build_nc()
    return _NC_CACHE


def _to_bf16(x: np.ndarray) -> np.ndarray:
    import ml_dtypes

    return np.asarray(x).astype(ml_dtypes.bfloat16)


def _host_shards(z_i: np.ndarray, z_j: np.ndarray):
    """-> (zpack_stack [8*NPACK, D],) bf16: per core, the full z followed by
    its own 512 sim rows and their 512 positive-pair rows."""
    z = np.concatenate([np.asarray(z_i), np.asarray(z_j)], axis=0)
    zbf = _to_bf16(z)
    zpos = np.roll(zbf, -B, axis=0)
    parts = []
    for c in range(NCORES):
        parts.append(zbf)
        parts.append(zbf[c * RPC : (c + 1) * RPC])
        parts.append(zpos[c * RPC : (c + 1) * RPC])
    return (np.concatenate(parts, axis=0),)


def make_in_maps(z_i: np.ndarray, z_j: np.ndarray):
    (zpack_stack,) = _host_shards(z_i, z_j)
    return [
        {"zpack": zpack_stack[c * NPACK : (c + 1) * NPACK]}
        for c in range(NCORES)
    ]


_EXEC_CACHE = None


def _get_exec():
    """Jitted 8-core SPMD executable (fast-dispatch compiled), built once."""
    global _EXEC_CACHE
    if _EXEC_CACHE is None:
        import jax
        from jax.experimental.shard_map import shard_map
        from jax.sharding import Mesh, PartitionSpec

        from concourse import bass2jax

        nc = _get_nc()
        bass2jax.install_neuronx_cc_hook()
        assert nc.dbg_addr is None
        part_name = (
            nc.partition_id_tensor.name if nc.partition_id_tensor else None
        )
        # input order: ExternalInputs, partition id.  The "out" tensor is
        # NOT passed as a donated zeroed operand (run_bass_via_pjrt does
        # that only to guarantee zero-init for kernels that don't write
        # every element — ours fully writes out[1,1]); binding it purely
        # as the custom-call result drops one operand + per-call donation
        # bookkeeping from the dispatch path.
        in_names = list(IN_NAMES) + ([part_name] if part_name else [])
        out_avals = (jax.core.ShapedArray(OUT_SHAPE, np.float32),)

        def _body(*args):
            operands = list(args)
            if part_name is not None:
                operands.append(bass2jax.partition_id_tensor())
            outs = bass2jax._bass_exec_p.bind(
                *operands,
                out_avals=out_avals,
                in_names=tuple(in_names),
                out_names=("out",),
                lowering_input_output_aliases=(),
                sim_require_finite=True,
                sim_require_nnan=True,
                nc=nc,
            )
            return tuple(outs)

        devices = jax.devices()[:NCORES]
        mesh = Mesh(np.asarray(devices), ("core",))
        n_in = len(IN_NAMES)

        def make_jit():
            return jax.jit(
                shard_map(
                    _body,
                    mesh=mesh,
                    in_specs=(PartitionSpec("core"),) * n_in,
                    out_specs=(PartitionSpec("core"),),
                    check_rep=False,
                ),
                keep_unused=True,
            )

        import ml_dtypes

        ex_args = (np.zeros((NCORES * NPACK, D), ml_dtypes.bfloat16),)
        try:
            # fast_dispatch_compile minus the FastDispatchCompiled reclass:
            # kernel() always reads its outputs, so the never-read-output
            # safety net would only add per-call host work.
            with bass2jax._fast_dispatch_active(True):
                compiled = make_jit().lower(*ex_args).compile()
            if compiled._executable.unsafe_call.has_unordered_effects:
                raise RuntimeError("bass_effect still present")
            _EXEC_CACHE = compiled
        except Exception:
            _EXEC_CACHE = make_jit()
    return _EXEC_CACHE


def kernel(z_i: np.ndarray, z_j: np.ndarray) -> np.ndarray:
    """Full inputs in, full output out; shards sim rows across the 8 cores."""
    sharded = _get_exec()
    (zpack_stack,) = _host_shards(z_i, z_j)
    (partials,) = sharded(zpack_stack)
    return np.float32(float(np.asarray(partials).sum()) / N)
